# revision 22
# baseline (speedup 1.0000x reference)
"""Trainium2 Bass kernel for the decoder loss (likelihood, kl).

Strategy: vocab-parallel across 8 NeuronCores. Core c owns vocab rows
[c*6250, (c+1)*6250) of W_e and W_f. Each core computes the partial
softmax denominators Z_e[t], Z_f[t] = sum_v exp(z_t . w_v) for all 1024
tokens over its vocab shard. The host sums partials across cores (the
"all-reduce" of the vocab-parallel softmax), takes logs, and computes
every O(T*D) selected-logit / KL term in float64.

Device pipeline (per core):
  - PE: fp8(e4m3) DoubleRow matmuls -- K=256 contracted in ONE pass
    (2 fp8 MACs/PE/cycle, 216ns per 512-col tile): ~45us of PE work.
  - The exp + row-sum of the 12.8M logits/core is split ~56:44 between:
      * ACT: exp with fused accum_out (per-token partial sums; the
        285ns accumulator drain beats any external reduce).
      * DVE: Schraudolph bit-trick exp -- tensor_scalar computes
        round(A*logit + B) into int16 whose bit pattern IS bf16
        exp(logit) (magic constant calibrated so E[approx/exact] = 1
        to 1e-9 under uniform mantissa phase; the +-1.8% RMS
        oscillation averages out over the 50k-term sums). Reduction:
        PAIRS of Schraudolph chunks are summed+reduced in a single
        two-stream scalar_tensor_tensor pass with fused accum_out
        (half the reduce cost); the pair's combined sum lands in one
        stats column (the partner column stays zero; the host sums
        all columns). Pairs never straddle the E/F matrix boundary.
  - Vocab is walked in 6 chunks of 1024 + a 106-col ragged tail per
    matrix, with a 4-deep PSUM ring ([128,1024] x 4 = all 8 banks), so
    the mm -> consumer -> mm buffer-turnaround latency hides behind
    three other chunks' consumer work instead of stalling the ring.
    The tails of both matrices run first, batched across all 8 token
    tiles in one PSUM tile, while the PE ramps.

Inputs are scaled (z*16, W*512) to keep fp8 in the normal range; exp
reads PSUM with scale=1/8192. fp8 noise only touches the averaged
denominators (logit noise ~1e-2 -> Z bias ~5e-5, well under tolerance).
"""

import numpy as np

B, S, SF, DIM = 16, 64, 48, 256
VE, VF = 50000, 50000
NCORES = 8
T = B * S  # 1024
VSH = VE // NCORES  # 6250 vocab rows per core per matrix
NT = T // 128  # 8 token tiles
CHUNK = 1024
NCH = 6  # 6 * 1024 = 6144; +106 ragged tail per matrix
TAIL = VSH - NCH * CHUNK  # 106

SCALE_Z = 16.0
SCALE_W = 512.0
SCALE = SCALE_Z * SCALE_W  # 8192
# bf16 Schraudolph: i16 = round(A*psum + B); bitcast(i16) ~ exp(psum/SCALE)
SCH_A = 128.0 * np.log2(np.e) / SCALE  # 0.022542110013890053
SCH_B = 16248.640741050927  # 128*127 - 7.3592589 (calibrated, mean-exact)

# token tiles handled by the DVE per phase; alternating 3/4 gives the
# ~44% DVE share that balances the two engines' rates
DVE_TTS = ((1, 4, 6), (0, 2, 5, 7))

_PROGRAM_CACHE = {}
LAST_RESULTS = None  # BassKernelResults of the most recent run (for profiling)


def _build_program(has_be: bool, has_bf: bool):
    import concourse.bass as bass  # noqa: F401
    import concourse.tile as tile
    from concourse import bacc, mybir

    f32 = mybir.dt.float32
    bf16 = mybir.dt.bfloat16
    i16 = mybir.dt.int16
    fp8 = mybir.dt.float8e4
    Exp = mybir.ActivationFunctionType.Exp
    DR = mybir.MatmulPerfMode.DoubleRow
    addop = mybir.AluOpType.add
    multop = mybir.AluOpType.mult
    X = mybir.AxisListType.X

    nc = bacc.Bacc(
        "TRN2",
        target_bir_lowering=False,
        debug=False,
        enable_asserts=False,
        num_devices=NCORES,
    )

    # --- I/O ---
    zt_d = nc.dram_tensor("zt", [2 * 128, T], fp8, kind="ExternalInput")
    wet_d = nc.dram_tensor("wet", [2 * 128, VSH], fp8, kind="ExternalInput")
    wft_d = nc.dram_tensor("wft", [2 * 128, VSH], fp8, kind="ExternalInput")
    beb_d = nc.dram_tensor("beb", [1, VSH], bf16, kind="ExternalInput") if has_be else None
    bfb_d = nc.dram_tensor("bfb", [1, VSH], bf16, kind="ExternalInput") if has_bf else None

    # stats: col = mat*56 + ci*8 + tt for full chunks, mat*56 + 48 + tt for
    # the ragged tails (paired DVE chunks write one col of the pair and
    # leave the other zero; the host sums everything)
    st_d = nc.dram_tensor("st", [128, 112], f32, kind="ExternalOutput")

    with tile.TileContext(nc) as tc:
        with (
            tc.tile_pool(name="const", bufs=1) as cpool,
            tc.tile_pool(name="wstream", bufs=3) as wpool,
            tc.tile_pool(name="escr", bufs=1) as epool,
            tc.tile_pool(name="sscr", bufs=3) as hpool,
            tc.tile_pool(name="dscr", bufs=1) as dpool,
            tc.tile_pool(name="stats", bufs=1) as stpool,
            tc.tile_pool(name="psum", bufs=4, space="PSUM") as ppool,
        ):
            # PE warmup: dense dummy matmuls with no input deps flip the HAM
            # clock gate to 2.4 GHz while the first DMAs are in flight; the
            # dummy exp pulls the ACT table load into the preamble.
            wk = cpool.tile([128, 16], bf16, tag="warm")
            nc.gpsimd.memset(wk[:, :], 1.0)
            wact = cpool.tile([1, 16], f32, tag="wact")
            nc.scalar.activation(wact[:, :], wk[0:1, 0:16], Exp)

            ones = None
            if has_be or has_bf:
                ones = cpool.tile([1, 128], bf16, tag="ones")
                nc.gpsimd.memset(ones[:, :], 1.0)

            zt = cpool.tile([128, 2, T], fp8, tag="zt")
            nc.sync.dma_start(zt[:, :, :], zt_d.rearrange("(k p) t -> p k t", k=2))

            st = stpool.tile([128, 112], f32, tag="st")
            nc.vector.memset(st[:, :], 0.0)

            mats = (
                (wet_d, beb_d, 0),
                (wft_d, bfb_d, 56),
            )

            # --- ragged tails first (both matrices), batched over token
            # tiles; runs while the PE ramps and the first chunks stream.
            for w_d, b_d, col0 in mats:
                wtl = cpool.tile([128, 2, TAIL], fp8, tag=f"wtl{col0}")
                nc.sync.dma_start(
                    wtl[:, :, :],
                    w_d.rearrange("(k p) v -> p k v", k=2)[:, :, NCH * CHUNK :],
                )
                btl = None
                if b_d is not None:
                    btl = cpool.tile([1, TAIL], bf16, tag=f"btl{col0}")
                    nc.sync.dma_start(btl[:, :], b_d[:, NCH * CHUNK :])
                # [128, 8, 128] padded so each tt slice is 512B-aligned
                pst = ppool.tile([128, NT, 128], f32, tag="ps")
                for tt in range(NT):
                    nc.tensor.matmul(
                        pst[:, tt, 0:TAIL],
                        zt[:, :, tt * 128 : (tt + 1) * 128],
                        wtl[:, :, :],
                        start=True,
                        stop=(b_d is None),
                        perf_mode=DR,
                    )
                    if b_d is not None:
                        nc.tensor.matmul(
                            pst[:, tt, 0:TAIL], ones[:, :], btl[:, :],
                            start=False, stop=True,
                        )
                ext = epool.tile([128, NT, TAIL], bf16, tag="ex")
                nc.scalar.activation(
                    ext[:, :, :], pst[:, :, 0:TAIL], Exp, scale=1.0 / SCALE
                )
                nc.vector.tensor_reduce(
                    st[:, col0 + 48 : col0 + 56], ext[:, :, :], X, addop
                )

            # --- main sweep ---
            phase = 0
            pending = None  # (sch_tile, col) awaiting its ttr partner
            for w_d, b_d, col0 in mats:
                for ci in range(NCH):
                    c0 = ci * CHUNK
                    wt = wpool.tile([128, 2, CHUNK], fp8, tag="w")
                    nc.sync.dma_start(
                        wt[:, :, :],
                        w_d.rearrange("(k p) v -> p k v", k=2)[:, :, c0 : c0 + CHUNK],
                    )
                    bt = None
                    if b_d is not None:
                        bt = wpool.tile([1, CHUNK], bf16, tag="b")
                        nc.sync.dma_start(bt[:, :], b_d[:, c0 : c0 + CHUNK])
                    dve_tts = DVE_TTS[phase % 2]
                    for tt in range(NT):
                        ps = ppool.tile([128, CHUNK], f32, tag="ps")
                        for n0 in range(0, CHUNK, 512):
                            n1 = min(CHUNK, n0 + 512)
                            nc.tensor.matmul(
                                ps[:, n0:n1],
                                zt[:, :, tt * 128 : (tt + 1) * 128],
                                wt[:, :, n0:n1],
                                start=True,
                                stop=(b_d is None),
                                perf_mode=DR,
                            )
                            if b_d is not None:
                                nc.tensor.matmul(
                                    ps[:, n0:n1], ones[:, :], bt[:, n0:n1],
                                    start=False, stop=True,
                                )
                        col = col0 + ci * 8 + tt
                        if tt in dve_tts:
                            sch = hpool.tile([128, CHUNK], i16, tag="sch")
                            nc.vector.tensor_scalar(
                                sch[:, :], ps[:, :], SCH_A, SCH_B, multop, addop
                            )
                            if pending is None:
                                pending = (sch, col)
                            else:
                                # two-stream pair reduce: one pass sums BOTH
                                # Schraudolph chunks into a single stats col
                                dummy = dpool.tile([128, CHUNK], bf16, tag="dum")
                                nc.vector.scalar_tensor_tensor(
                                    dummy[:, :],
                                    pending[0][:, :].bitcast(bf16), 0.0,
                                    sch[:, :].bitcast(bf16),
                                    addop, addop,
                                    accum_out=st[:, pending[1] : pending[1] + 1],
                                )
                                pending = None
                        else:
                            ex = epool.tile([128, CHUNK], bf16, tag="ex")
                            nc.scalar.activation(
                                ex[:, :], ps[:, :], Exp, scale=1.0 / SCALE,
                                accum_out=st[:, col : col + 1],
                            )
                    phase += 1
                # flush the unpaired chunk at the matrix boundary: a pair
                # must never mix Z_e and Z_f columns
                if pending is not None:
                    nc.vector.tensor_reduce(
                        st[:, pending[1] : pending[1] + 1],
                        pending[0][:, :].bitcast(bf16), X, addop,
                    )
                    pending = None
            nc.sync.dma_start(st_d[:, :], st[:, :])

    nc.compile()
    return nc


def _get_program(has_be: bool, has_bf: bool):
    key = (has_be, has_bf)
    if key not in _PROGRAM_CACHE:
        _PROGRAM_CACHE[key] = _build_program(has_be, has_bf)
    return _PROGRAM_CACHE[key]


def kernel(mu_l, sigma_l, english, french, W_e, b_e, W_f, b_f):
    global LAST_RESULTS
    import os

    if os.environ.get("BASS_TRACE"):
        # tracing under axon needs the antenv.axon_hooks glue; disable
        # tracing rather than crash if it is absent (grading environments).
        try:
            import antenv.axon_hooks  # noqa: F401
        except ImportError:
            os.environ["BASS_NEVER_TRACE"] = "1"
    from concourse.bass_utils import run_bass_kernel_spmd

    import ml_dtypes

    fp8 = ml_dtypes.float8_e4m3fn
    bf16 = ml_dtypes.bfloat16

    mu = np.asarray(mu_l, dtype=np.float32).reshape(T, DIM)
    sg = np.asarray(sigma_l, dtype=np.float32).reshape(T, DIM)
    eng = np.asarray(english).reshape(T).astype(np.int64)
    fr = np.asarray(french).reshape(B, SF).astype(np.int64)
    We = np.ascontiguousarray(np.asarray(W_e, dtype=np.float32))
    Wf = np.ascontiguousarray(np.asarray(W_f, dtype=np.float32))
    be = np.asarray(b_e, dtype=np.float32).reshape(VE)
    bf = np.asarray(b_f, dtype=np.float32).reshape(VF)
    has_be = bool(be.any())
    has_bf = bool(bf.any())

    z = mu + sg  # [1024, 256] fp32, same as reference
    zT8 = np.clip(z.T * SCALE_Z, -240, 240).astype(fp8)  # [256, 1024]
    zT8 = np.ascontiguousarray(zT8)

    nc = _get_program(has_be, has_bf)

    in_maps = []
    for c in range(NCORES):
        vs = slice(c * VSH, (c + 1) * VSH)
        m = {
            "zt": zT8,
            "wet": np.ascontiguousarray(
                np.clip(We[vs].T * SCALE_W, -240, 240).astype(fp8)
            ),
            "wft": np.ascontiguousarray(
                np.clip(Wf[vs].T * SCALE_W, -240, 240).astype(fp8)
            ),
        }
        if has_be:
            m["beb"] = np.ascontiguousarray(
                (be[vs] * SCALE).reshape(1, VSH)
            ).astype(bf16)
        if has_bf:
            m["bfb"] = np.ascontiguousarray(
                (bf[vs] * SCALE).reshape(1, VSH)
            ).astype(bf16)
        in_maps.append(m)

    LAST_RESULTS = run_bass_kernel_spmd(nc, in_maps, list(range(NCORES)))
    res = LAST_RESULTS.results

    # --- host finalize: sum partial Z across cores, all O(T*D) terms ---
    st_all = np.zeros((128, 112), dtype=np.float64)
    for c in range(NCORES):
        st_all += res[c]["st"].astype(np.float64)
    # col = mat*56 + ci*8 + tt ; token t = tt*128 + partition
    Ze = st_all[:, 0:56].reshape(128, NCH + 1, NT).sum(1).T.ravel()  # [1024]
    Zf = st_all[:, 56:112].reshape(128, NCH + 1, NT).sum(1).T.ravel()

    z64 = z.astype(np.float64)
    # English: sum_t [ z_t . We[eng_t] + be[eng_t] - lse_t ]
    dots = np.einsum("td,td->t", z64, We[eng].astype(np.float64))
    lse = np.log(Ze)
    Le = dots.sum() + be[eng].astype(np.float64).sum() - lse.sum()

    # French: sel_pf[b,k] = mean_s exp(z_bs . Wf[fr_bk] + bf[fr_bk]) / Zf[b,s]
    zb = z64.reshape(B, S, DIM)
    Wg = Wf[fr].astype(np.float64)  # [B, SF, DIM]
    logits_sel = np.einsum("bsd,bkd->bsk", zb, Wg) + bf[fr].astype(np.float64)[
        :, None, :
    ]
    num = np.exp(logits_sel)  # [B, S, SF]
    selpf = (num / Zf.reshape(B, S)[:, :, None]).mean(axis=1)  # [B, SF]
    likelihood = Le + np.log(selpf).sum()

    # KL(N(mu, sigma) || N(0,1)) summed over all elements
    mu64 = mu.astype(np.float64)
    sg64 = sg.astype(np.float64)
    kl = (-np.log(sg64) + 0.5 * (sg64**2 + mu64**2)).sum() - 0.5 * mu64.size
    return (np.float32(likelihood), np.float32(kl))


# revision 23
# speedup vs baseline: 1.2584x; 1.2584x over previous
"""Trainium2 Bass kernel for the decoder loss (likelihood, kl).

Strategy: vocab-parallel across 8 NeuronCores. Core c owns vocab rows
[c*6250, (c+1)*6250) of W_e and W_f. Each core computes the partial
softmax denominators Z_e[t], Z_f[t] = sum_v exp(z_t . w_v) for all 1024
tokens over its vocab shard. The host sums partials across cores (the
"all-reduce" of the vocab-parallel softmax), takes logs, and computes
every O(T*D) selected-logit / KL term in float64.

Device pipeline (per core):
  - PE: fp8(e4m3) DoubleRow matmuls -- K=256 contracted in ONE pass
    (2 fp8 MACs/PE/cycle, 216ns per 512-col tile): ~45us of PE work.
  - The exp + row-sum of the 12.8M logits/core is split ~56:44 between:
      * ACT: exp with fused accum_out (per-token partial sums; the
        285ns accumulator drain beats any external reduce).
      * DVE: Schraudolph bit-trick exp -- tensor_scalar computes
        round(A*logit + B) into int16 whose bit pattern IS bf16
        exp(logit) (magic constant calibrated so E[approx/exact] = 1
        to 1e-9 under uniform mantissa phase; the +-1.8% RMS
        oscillation averages out over the 50k-term sums). Reduction:
        PAIRS of Schraudolph chunks are summed+reduced in a single
        two-stream scalar_tensor_tensor pass with fused accum_out
        (half the reduce cost); the pair's combined sum lands in one
        stats column (the partner column stays zero; the host sums
        all columns). Pairs never straddle the E/F matrix boundary.
  - Vocab is walked in 6 chunks of 1024 + a 106-col ragged tail per
    matrix, with a 4-deep PSUM ring ([128,1024] x 4 = all 8 banks), so
    the mm -> consumer -> mm buffer-turnaround latency hides behind
    three other chunks' consumer work instead of stalling the ring.
    The tails of both matrices run first, batched across all 8 token
    tiles in one PSUM tile, while the PE ramps.

Inputs are scaled (z*16, W*512) to keep fp8 in the normal range; exp
reads PSUM with scale=1/8192. fp8 noise only touches the averaged
denominators (logit noise ~1e-2 -> Z bias ~5e-5, well under tolerance).
"""

import numpy as np

B, S, SF, DIM = 16, 64, 48, 256
VE, VF = 50000, 50000
NCORES = 8
T = B * S  # 1024
VSH = VE // NCORES  # 6250 vocab rows per core per matrix
NT = T // 128  # 8 token tiles
CHUNK = 1024
NCH = 6  # 6 * 1024 = 6144; +106 ragged tail per matrix
TAIL = VSH - NCH * CHUNK  # 106

SCALE_Z = 16.0
SCALE_W = 512.0
SCALE = SCALE_Z * SCALE_W  # 8192
# bf16 Schraudolph: i16 = round(A*psum + B); bitcast(i16) ~ exp(psum/SCALE)
SCH_A = 128.0 * np.log2(np.e) / SCALE  # 0.022542110013890053
SCH_B = 16248.640741050927  # 128*127 - 7.3592589 (calibrated, mean-exact)

# token tiles handled by the DVE per phase; alternating 3/4 gives the
# ~44% DVE share that balances the two engines' rates
DVE_TTS = ((1, 4, 6), (0, 2, 5, 7))

_PROGRAM_CACHE = {}
LAST_RESULTS = None  # BassKernelResults of the most recent run (for profiling)


def _build_program(has_be: bool, has_bf: bool):
    import concourse.bass as bass  # noqa: F401
    import concourse.tile as tile
    from concourse import bacc, mybir

    f32 = mybir.dt.float32
    bf16 = mybir.dt.bfloat16
    i16 = mybir.dt.int16
    fp8 = mybir.dt.float8e4
    Exp = mybir.ActivationFunctionType.Exp
    DR = mybir.MatmulPerfMode.DoubleRow
    addop = mybir.AluOpType.add
    multop = mybir.AluOpType.mult
    X = mybir.AxisListType.X

    nc = bacc.Bacc(
        "TRN2",
        target_bir_lowering=False,
        debug=False,
        enable_asserts=False,
        num_devices=NCORES,
    )

    # --- I/O ---
    zt_d = nc.dram_tensor("zt", [2 * 128, T], fp8, kind="ExternalInput")
    wet_d = nc.dram_tensor("wet", [2 * 128, VSH], fp8, kind="ExternalInput")
    wft_d = nc.dram_tensor("wft", [2 * 128, VSH], fp8, kind="ExternalInput")
    beb_d = nc.dram_tensor("beb", [1, VSH], bf16, kind="ExternalInput") if has_be else None
    bfb_d = nc.dram_tensor("bfb", [1, VSH], bf16, kind="ExternalInput") if has_bf else None

    # stats: col = mat*56 + ci*8 + tt for full chunks, mat*56 + 48 + tt for
    # the ragged tails (paired DVE chunks write one col of the pair and
    # leave the other zero; the host sums everything)
    st_d = nc.dram_tensor("st", [128, 112], f32, kind="ExternalOutput")

    with tile.TileContext(nc) as tc:
        with (
            tc.tile_pool(name="const", bufs=1) as cpool,
            tc.tile_pool(name="wstream", bufs=3) as wpool,
            tc.tile_pool(name="escr", bufs=2) as epool,
            tc.tile_pool(name="sscr", bufs=4) as hpool,
            tc.tile_pool(name="dscr", bufs=2) as dpool,
            tc.tile_pool(name="stats", bufs=1) as stpool,
            tc.tile_pool(name="psum", bufs=4, space="PSUM") as ppool,
        ):
            # PE warmup: dense dummy matmuls with no input deps flip the HAM
            # clock gate to 2.4 GHz while the first DMAs are in flight; the
            # dummy exp pulls the ACT table load into the preamble.
            wk = cpool.tile([128, 512], bf16, tag="warm")
            nc.gpsimd.memset(wk[:, :], 1.0)
            wact = cpool.tile([1, 16], f32, tag="wact")
            nc.scalar.activation(wact[:, :], wk[0:1, 0:16], Exp)
            wps = ppool.tile([128, 512], f32, tag="ps")
            for wi in range(14):
                nc.tensor.matmul(
                    wps[:, :], wk[:, 0:128], wk[:, :], start=True, stop=True
                )

            ones = None
            if has_be or has_bf:
                ones = cpool.tile([1, 128], bf16, tag="ones")
                nc.gpsimd.memset(ones[:, :], 1.0)

            zt = cpool.tile([128, 2, T], fp8, tag="zt")
            nc.sync.dma_start(zt[:, :, :], zt_d.rearrange("(k p) t -> p k t", k=2))

            st = stpool.tile([128, 112], f32, tag="st")
            nc.vector.memset(st[:, :], 0.0)

            mats = (
                (wet_d, beb_d, 0),
                (wft_d, bfb_d, 56),
            )

            # --- ragged tails first (both matrices), batched over token
            # tiles; runs while the PE ramps and the first chunks stream.
            for w_d, b_d, col0 in mats:
                wtl = cpool.tile([128, 2, TAIL], fp8, tag=f"wtl{col0}")
                nc.sync.dma_start(
                    wtl[:, :, :],
                    w_d.rearrange("(k p) v -> p k v", k=2)[:, :, NCH * CHUNK :],
                )
                btl = None
                if b_d is not None:
                    btl = cpool.tile([1, TAIL], bf16, tag=f"btl{col0}")
                    nc.sync.dma_start(btl[:, :], b_d[:, NCH * CHUNK :])
                # [128, 8, 128] padded so each tt slice is 512B-aligned
                pst = ppool.tile([128, NT, 128], f32, tag="ps")
                for tt in range(NT):
                    nc.tensor.matmul(
                        pst[:, tt, 0:TAIL],
                        zt[:, :, tt * 128 : (tt + 1) * 128],
                        wtl[:, :, :],
                        start=True,
                        stop=(b_d is None),
                        perf_mode=DR,
                    )
                    if b_d is not None:
                        nc.tensor.matmul(
                            pst[:, tt, 0:TAIL], ones[:, :], btl[:, :],
                            start=False, stop=True,
                        )
                ext = epool.tile([128, NT, TAIL], bf16, tag="ex")
                nc.scalar.activation(
                    ext[:, :, :], pst[:, :, 0:TAIL], Exp, scale=1.0 / SCALE
                )
                nc.vector.tensor_reduce(
                    st[:, col0 + 48 : col0 + 56], ext[:, :, :], X, addop
                )

            # --- main sweep ---
            phase = 0
            pending = None  # (sch_tile, col) awaiting its ttr partner
            for w_d, b_d, col0 in mats:
                for ci in range(NCH):
                    c0 = ci * CHUNK
                    wt = wpool.tile([128, 2, CHUNK], fp8, tag="w")
                    nc.sync.dma_start(
                        wt[:, :, :],
                        w_d.rearrange("(k p) v -> p k v", k=2)[:, :, c0 : c0 + CHUNK],
                    )
                    bt = None
                    if b_d is not None:
                        bt = wpool.tile([1, CHUNK], bf16, tag="b")
                        nc.sync.dma_start(bt[:, :], b_d[:, c0 : c0 + CHUNK])
                    dve_tts = DVE_TTS[phase % 2]
                    for tt in range(NT):
                        ps = ppool.tile([128, CHUNK], f32, tag="ps")
                        for n0 in range(0, CHUNK, 512):
                            n1 = min(CHUNK, n0 + 512)
                            nc.tensor.matmul(
                                ps[:, n0:n1],
                                zt[:, :, tt * 128 : (tt + 1) * 128],
                                wt[:, :, n0:n1],
                                start=True,
                                stop=(b_d is None),
                                perf_mode=DR,
                            )
                            if b_d is not None:
                                nc.tensor.matmul(
                                    ps[:, n0:n1], ones[:, :], bt[:, n0:n1],
                                    start=False, stop=True,
                                )
                        col = col0 + ci * 8 + tt
                        if tt in dve_tts:
                            sch = hpool.tile([128, CHUNK], i16, tag="sch")
                            nc.vector.tensor_scalar(
                                sch[:, :], ps[:, :], SCH_A, SCH_B, multop, addop
                            )
                            if pending is None:
                                pending = (sch, col)
                            else:
                                # two-stream pair reduce: one pass sums BOTH
                                # Schraudolph chunks into a single stats col
                                dummy = dpool.tile([128, CHUNK], bf16, tag="dum")
                                nc.vector.scalar_tensor_tensor(
                                    dummy[:, :],
                                    pending[0][:, :].bitcast(bf16), 0.0,
                                    sch[:, :].bitcast(bf16),
                                    addop, addop,
                                    accum_out=st[:, pending[1] : pending[1] + 1],
                                )
                                pending = None
                        else:
                            ex = epool.tile([128, CHUNK], bf16, tag="ex")
                            nc.scalar.activation(
                                ex[:, :], ps[:, :], Exp, scale=1.0 / SCALE,
                                accum_out=st[:, col : col + 1],
                            )
                    phase += 1
                # flush the unpaired chunk at the matrix boundary: a pair
                # must never mix Z_e and Z_f columns
                if pending is not None:
                    nc.vector.tensor_reduce(
                        st[:, pending[1] : pending[1] + 1],
                        pending[0][:, :].bitcast(bf16), X, addop,
                    )
                    pending = None
            nc.sync.dma_start(st_d[:, :], st[:, :])

    nc.compile()
    return nc


def _get_program(has_be: bool, has_bf: bool):
    key = (has_be, has_bf)
    if key not in _PROGRAM_CACHE:
        _PROGRAM_CACHE[key] = _build_program(has_be, has_bf)
    return _PROGRAM_CACHE[key]


def kernel(mu_l, sigma_l, english, french, W_e, b_e, W_f, b_f):
    global LAST_RESULTS
    import os

    if os.environ.get("BASS_TRACE"):
        # tracing under axon needs the antenv.axon_hooks glue; disable
        # tracing rather than crash if it is absent (grading environments).
        try:
            import antenv.axon_hooks  # noqa: F401
        except ImportError:
            os.environ["BASS_NEVER_TRACE"] = "1"
    from concourse.bass_utils import run_bass_kernel_spmd

    import ml_dtypes

    fp8 = ml_dtypes.float8_e4m3fn
    bf16 = ml_dtypes.bfloat16

    mu = np.asarray(mu_l, dtype=np.float32).reshape(T, DIM)
    sg = np.asarray(sigma_l, dtype=np.float32).reshape(T, DIM)
    eng = np.asarray(english).reshape(T).astype(np.int64)
    fr = np.asarray(french).reshape(B, SF).astype(np.int64)
    We = np.ascontiguousarray(np.asarray(W_e, dtype=np.float32))
    Wf = np.ascontiguousarray(np.asarray(W_f, dtype=np.float32))
    be = np.asarray(b_e, dtype=np.float32).reshape(VE)
    bf = np.asarray(b_f, dtype=np.float32).reshape(VF)
    has_be = bool(be.any())
    has_bf = bool(bf.any())

    z = mu + sg  # [1024, 256] fp32, same as reference
    zT8 = np.clip(z.T * SCALE_Z, -240, 240).astype(fp8)  # [256, 1024]
    zT8 = np.ascontiguousarray(zT8)

    nc = _get_program(has_be, has_bf)

    in_maps = []
    for c in range(NCORES):
        vs = slice(c * VSH, (c + 1) * VSH)
        m = {
            "zt": zT8,
            "wet": np.ascontiguousarray(
                np.clip(We[vs].T * SCALE_W, -240, 240).astype(fp8)
            ),
            "wft": np.ascontiguousarray(
                np.clip(Wf[vs].T * SCALE_W, -240, 240).astype(fp8)
            ),
        }
        if has_be:
            m["beb"] = np.ascontiguousarray(
                (be[vs] * SCALE).reshape(1, VSH)
            ).astype(bf16)
        if has_bf:
            m["bfb"] = np.ascontiguousarray(
                (bf[vs] * SCALE).reshape(1, VSH)
            ).astype(bf16)
        in_maps.append(m)

    LAST_RESULTS = run_bass_kernel_spmd(nc, in_maps, list(range(NCORES)))
    res = LAST_RESULTS.results

    # --- host finalize: sum partial Z across cores, all O(T*D) terms ---
    st_all = np.zeros((128, 112), dtype=np.float64)
    for c in range(NCORES):
        st_all += res[c]["st"].astype(np.float64)
    # col = mat*56 + ci*8 + tt ; token t = tt*128 + partition
    Ze = st_all[:, 0:56].reshape(128, NCH + 1, NT).sum(1).T.ravel()  # [1024]
    Zf = st_all[:, 56:112].reshape(128, NCH + 1, NT).sum(1).T.ravel()

    z64 = z.astype(np.float64)
    # English: sum_t [ z_t . We[eng_t] + be[eng_t] - lse_t ]
    dots = np.einsum("td,td->t", z64, We[eng].astype(np.float64))
    lse = np.log(Ze)
    Le = dots.sum() + be[eng].astype(np.float64).sum() - lse.sum()

    # French: sel_pf[b,k] = mean_s exp(z_bs . Wf[fr_bk] + bf[fr_bk]) / Zf[b,s]
    zb = z64.reshape(B, S, DIM)
    Wg = Wf[fr].astype(np.float64)  # [B, SF, DIM]
    logits_sel = np.einsum("bsd,bkd->bsk", zb, Wg) + bf[fr].astype(np.float64)[
        :, None, :
    ]
    num = np.exp(logits_sel)  # [B, S, SF]
    selpf = (num / Zf.reshape(B, S)[:, :, None]).mean(axis=1)  # [B, SF]
    likelihood = Le + np.log(selpf).sum()

    # KL(N(mu, sigma) || N(0,1)) summed over all elements
    mu64 = mu.astype(np.float64)
    sg64 = sg.astype(np.float64)
    kl = (-np.log(sg64) + 0.5 * (sg64**2 + mu64**2)).sum() - 0.5 * mu64.size
    return (np.float32(likelihood), np.float32(kl))
